# revision 51
# baseline (speedup 1.0000x reference)
"""DeltaAttention Trainium2 kernel — 8-core SPMD via bass/Tile.

Math (per reference): 4 DeltaResidualBlocks (d_v=1) wrapped around MHA.
Because each delta block consumes its v_in only through the scalar
projection v_in @ dWv[i], the Wq/Wk/Wv/Wo matmuls collapse into single
extra columns of the dWk matmuls (precomputed on host), and attn@v
collapses to 2 output columns per head:
    n_h[q] = E_h[q,:] @ u_h,  r_h[q] = E_h[q,:] @ 1,  u_h = v_h @ w_h
    v3[q]  = sum_h n_h/r_h + const,   w = Wo @ dWv[3]
Sharding: 512 query tokens per core; k^T and u are AllGathered within
each 4-core batch group.  Delta-block k_proj matmuls run in fp8
DoubleRow.  Restructured for early attention start: k path ships first,
then v/u (u gather), then q; the softmax exp stream (ScalarE-bound)
starts as soon as kT+qT are ready, with delta3 + moment math
interleaved into the attention loop's tensor/vector slack.  A subset of
exp tiles run on the Vector engine via a Schraudolph int16/bf16 bitcast
exp to unload ScalarE.  LayerNorm statistics come from precomputed
moments; sum(x) and sum(k3_raw) ride as extra columns of the extras
matmul.
"""

from contextlib import ExitStack

import numpy as np
import ml_dtypes

import concourse.bass as bass
import concourse.mybir as mybir
import concourse.tile as tile
from concourse.bass_utils import run_bass_kernel_spmd
from concourse.masks import make_identity

dt = mybir.dt
AF = mybir.ActivationFunctionType
ALU = mybir.AluOpType
PM = mybir.MatmulPerfMode
ts = bass.ts

N_CORES = 8
B, S, D, H = 2, 2048, 1024, 16
HD = D // H
TOK = (B * S) // N_CORES          # 512 query tokens per core
M4 = TOK // 128                   # 4 token chunks
K8 = D // 128                     # 8 feature chunks
G4 = D // 256                     # 4 double-row contraction groups
NKC = S // 128                    # 16 key chunks per batch
EPS = 1e-8
LN_EPS = 1e-5
AUG_SCALE = 64.0                  # fp8 weight scale; cancels in the algebra

AGD = D + 2 * H                   # k^T rows + u rides as 32 fp8 byte-rows

# extras matmul columns:
# [dbw0,vw0, dbw1,vw1, dbw2,vw2, Wu(16), Bu(16), dbw3, ones, mk3]
W_EX = 41
EX_DBW = [0, 2, 4, 38]
EX_VW = [1, 3, 5]
EX_A = 6      # 6..22  = x @ Wu
EX_B = 22     # 22..38 = x @ (AUG_SCALE*dWk2) @ Wu
EX_MX = 39    # x @ ones            -> sum_f x   (for LN mean)
EX_MK3 = 40   # x @ (AUG*dWk3)@ones -> sum_f k3raw_scaled

SC_DT = mybir.dt.float8e4   # q^T/k^T dtype for the scores matmul
SCALE = float(HD) ** -0.5

# Schraudolph exp on DVE: E = bitcast_bf16(int16(A*s + B)) ~= exp(s*SCALE)
LOG2E = 1.4426950408889634
SCHRA_A = 128.0 * LOG2E * SCALE
SCHRA_B = 128.0 * (127.0 - 0.04303)
DVE_POS = tuple(range(6, 16))     # kc-order positions where head1's exp
                                  # half runs on VectorE (head0 on ScalarE)

LAST_RESULTS = None
_CACHE = {}
DEBUG = False


def _split_multi_waits(nc, max_waits=1):
    """walrus (CoreV3) only encodes one sync wait per instruction; Tile's
    final drain can carry several. Hoist extras onto preceding NoOps."""
    n_fixed = 0
    for f in nc.m.functions:
        for blk in f.blocks:
            new_insts = []
            for inst in blk.instructions:
                si = inst.sync_info
                waits = list(si.on_wait) if (si and si.on_wait) else []
                if len(waits) > max_waits:
                    head, tail = waits[:-max_waits], waits[-max_waits:]
                    for j, w in enumerate(head):
                        nop = mybir.InstNoOp(
                            name=f"{inst.name}_waitsplit_{j}",
                            engine=inst.engine,
                            ins=[],
                            outs=[],
                            sync_info=mybir.SyncInfo(on_wait=[w], on_update=[]),
                        )
                        nc.register_instruction(nop)
                        new_insts.append(nop)
                        n_fixed += 1
                    si.on_wait = tail
                new_insts.append(inst)
            blk.instructions[:] = new_insts
    return n_fixed


def _build_program():
    nc = bass.Bass(num_devices=N_CORES)

    x_t = nc.dram_tensor("x", [TOK, D], dt.float32, kind="ExternalInput")
    # fp8 double-row layouts: [128, G4, 2, D]; [p, g, j, f] = W[256g+128j+p, f]
    aug_t = [
        nc.dram_tensor(f"aug{i}", [128, G4, 2, D], dt.float8e4, kind="ExternalInput")
        for i in range(4)
    ]
    ex_t = nc.dram_tensor("ex", [128, G4, 2, W_EX], dt.float8e4, kind="ExternalInput")
    exds_t = nc.dram_tensor("exds", [128, W_EX], dt.float32, kind="ExternalInput")
    cvec_t = nc.dram_tensor("cvec", [128, 16], dt.float32, kind="ExternalInput")
    lng_t = nc.dram_tensor("lng", [128, D], dt.bfloat16, kind="ExternalInput")
    lnb_t = nc.dram_tensor("lnb", [128, D], dt.bfloat16, kind="ExternalInput")
    y_t = nc.dram_tensor("y", [TOK, D], dt.bfloat16, kind="ExternalOutput")
    if DEBUG:
        dbg_u = nc.dram_tensor("dbg_u", [128, NKC, H], dt.bfloat16, kind="ExternalOutput")
        dbg_k = nc.dram_tensor("dbg_k", [128, S], dt.float32, kind="ExternalOutput")

    RG = [[0, 1, 2, 3], [4, 5, 6, 7]]

    with tile.TileContext(nc) as tc, ExitStack() as stack:
        const = stack.enter_context(tc.tile_pool(name="const", bufs=1))
        dram = stack.enter_context(tc.tile_pool(name="dram", bufs=1, space="DRAM"))
        big = stack.enter_context(tc.tile_pool(name="big", bufs=1))

        # ---- warmup collective first: its trigger only needs a tiny DMA, so
        # the all-core rendezvous barrier + CC stream setup run while the
        # real inputs load and the delta phase computes.
        warm_in = dram.tile([128, 4], dt.bfloat16, tag="warm_in")
        warm_out = dram.tile([4 * 128, 4], dt.bfloat16, tag="warm_out")
        warm_sb = const.tile([128, 4], dt.bfloat16, tag="wsb", name="warm_sb")
        nc.vector.memset(warm_sb[:], 0.0)
        nc.sync.dma_start(warm_in[:], warm_sb[:])
        nc.gpsimd.collective_compute(
            "AllGather", ALU.bypass, ins=[warm_in[:]], outs=[warm_out[:]],
            replica_groups=RG,
        )

        # ---- aug1 (gates the k projections) loads before x, then the
        # rest in use order
        wpool = stack.enter_context(tc.tile_pool(name="wpool", bufs=4))
        augsbs = {}
        aug1 = wpool.tile([128, G4, 2, D], dt.float8e4, tag="aug", name="augsb_1")
        nc.sync.dma_start(aug1[:], aug_t[1][:])
        augsbs[1] = aug1
        ext = wpool.tile([128, G4, 2, W_EX], dt.float8e4, tag="ext", name="ext")
        exds = const.tile([128, W_EX], dt.float32, tag="exds")
        cvec = const.tile([128, 16], dt.float32, tag="cvec")
        for i in (2, 0, 3):
            t = wpool.tile([128, G4, 2, D], dt.float8e4, tag="aug", name=f"augsb_{i}")
            augsbs[i] = t
        lng = const.tile([128, D], dt.bfloat16, tag="lng")
        lnb = const.tile([128, D], dt.bfloat16, tag="lnb")

        ident_bf = const.tile([128, 128], dt.bfloat16, tag="ident_bf")
        make_identity(nc, ident_bf[:])
        ident_f32 = const.tile([128, 128], dt.float32, tag="ident_f32")
        make_identity(nc, ident_f32[:])

        # collective staging (DRAM): the kT gather is split in halves so
        # attention can start on half A while B is in flight; u rides in B
        # as 4H byte-rows (bitcast bf16) of 128 tokens each.
        HTOK = TOK // 2
        agkA_in = dram.tile([D, HTOK], SC_DT, tag="agkA_in")
        agkA_out = dram.tile([4 * D, HTOK], SC_DT, tag="agkA_out")
        BGD = D + 4 * H
        agkB_in = dram.tile([BGD, HTOK], SC_DT, tag="agkB_in")
        agkB_out = dram.tile([4 * BGD, HTOK], SC_DT, tag="agkB_out")

        # persistent data tiles
        xbf = [big.tile([128, D], dt.bfloat16, tag=f"xbf_{m}", name=f"xbf_{m}") for m in range(M4)]
        xg = [big.tile([128, D], dt.bfloat16, tag=f"xg_{m}", name=f"xg_{m}") for m in range(M4)]
        xT8 = big.tile([128, K8, TOK], SC_DT, tag="xT8")
        qT = big.tile([128, K8, TOK], SC_DT, tag="qT")

        k3raw = [big.tile([128, D], dt.bfloat16, tag=f"k3_{m}", name=f"k3_{m}") for m in range(M4)]
        k3g = [big.tile([128, D], dt.bfloat16, tag=f"k3g_{m}", name=f"k3g_{m}") for m in range(M4)]
        a3s = big.tile([128, M4], dt.float32, tag="a3s")
        b3s = big.tile([128, M4], dt.float32, tag="b3s")
        u_bf = [big.tile([128, H], dt.bfloat16, tag=f"u_{m}", name=f"u_{m}") for m in range(M4)]
        exsb = [big.tile([128, W_EX], dt.float32, tag=f"ex_{m}", name=f"ex_{m}") for m in range(M4)]
        v3acc = big.tile([128, M4], dt.float32, tag="v3acc")
        mxs = big.tile([128, M4], dt.float32, tag="mxs")
        xxs = big.tile([128, M4], dt.float32, tag="xxs")
        mks3 = big.tile([128, M4], dt.float32, tag="mks3")
        kks3 = big.tile([128, M4], dt.float32, tag="kks3")
        xks3 = big.tile([128, M4], dt.float32, tag="xks3")

        nc.vector.memset(v3acc[:], 0.0)

        _n = [0]

        def sct(pool, name):
            _n[0] += 1
            return pool.tile([128, M4], dt.float32, tag="sc", name=f"{name}_{_n[0]}")

        def chain(i, ss, kx, sl, pool):
            """scalar chain on columns `sl` of [128, M4] tiles.
            Returns (rk, rr, s)."""
            n = sl.stop - sl.start
            exb = sct(pool, "exb")
            for m in range(sl.start, sl.stop):
                nc.vector.tensor_copy(
                    exb[:, m:m + 1], exsb[m][:, EX_DBW[i]:EX_DBW[i] + 1]
                )
            lnv, nrm, nrme, rnorm = (sct(pool, x) for x in ("lnv", "nrm", "nrme", "rn"))
            nc.scalar.activation(lnv[:, :n], ss[:, sl], AF.Ln)
            nc.scalar.activation(nrm[:, :n], lnv[:, :n], AF.Exp, scale=0.5)
            nc.vector.tensor_scalar_add(nrme[:, :n], nrm[:, :n], EPS * AUG_SCALE)
            nc.vector.reciprocal(rnorm[:, :n], nrme[:, :n])
            ez, ez1, rsig = (sct(pool, x) for x in ("ez", "ez1", "rs"))
            nc.scalar.activation(
                ez[:, :n], exb[:, sl], AF.Exp, scale=-1.0, bias=cvec[:, i:i + 1]
            )
            nc.vector.tensor_scalar(ez1[:, :n], ez[:, :n], 1.0, 0.5, ALU.add, ALU.mult)
            nc.vector.reciprocal(rsig[:, :n], ez1[:, :n])   # = 2*sigmoid
            rk, rr = sct(pool, "rk"), sct(pool, "rr")
            nc.vector.tensor_tensor(rk[:, :n], kx[:, sl], rnorm[:, :n], ALU.mult)
            nc.vector.tensor_tensor(rr[:, :n], rsig[:, :n], rnorm[:, :n], ALU.mult)
            if i == 3:
                return rk, rr, None
            v, dv, s = sct(pool, "v"), sct(pool, "dv"), sct(pool, "s")
            for j, m in enumerate(range(sl.start, sl.stop)):
                nc.vector.tensor_scalar_add(
                    v[:, j:j + 1], exsb[m][:, EX_VW[i]:EX_VW[i] + 1],
                    cvec[:, 4 + i:5 + i],
                )
            nc.vector.tensor_tensor(dv[:, :n], v[:, :n], rk[:, :n], ALU.subtract)
            nc.vector.tensor_tensor(s[:, :n], dv[:, :n], rr[:, :n], ALU.mult)
            return rk, rr, s

        # =========== phase 1: k path -> gather k; v/u path -> gather u;
        # =========== q path -> qT.  (attention pools open afterwards)
        with (
            tc.tile_pool(name="p1pool", bufs=1) as p1pool,
            tc.tile_pool(name="qkpool", bufs=8) as qkpool,
            tc.tile_pool(name="kspool", bufs=4) as kspool,
            tc.tile_pool(name="scpool", bufs=40) as scpool,
            tc.tile_pool(name="scr", bufs=4) as scrpool,
            tc.tile_pool(name="pp_proj", bufs=2, space="PSUM") as pp_proj,
            tc.tile_pool(name="pp_ex", bufs=2, space="PSUM") as pp_ex,
            tc.tile_pool(name="pp_t", bufs=2, space="PSUM") as pp_t,
        ):
            x32 = [p1pool.tile([128, D], dt.float32, tag=f"x32_{m}", name=f"x32_{m}") for m in range(M4)]
            for m in range(M4):
                nc.sync.dma_start(x32[m][:], x_t[ts(m, 128), :])
            nc.sync.dma_start(ext[:], ex_t[:])
            nc.sync.dma_start(exds[:], exds_t[:])
            nc.sync.dma_start(cvec[:], cvec_t[:])
            nc.sync.dma_start(augsbs[2][:], aug_t[2][:])
            nc.sync.dma_start(augsbs[0][:], aug_t[0][:])
            nc.sync.dma_start(augsbs[3][:], aug_t[3][:])
            nc.sync.dma_start(lng[:], lng_t[:])
            nc.sync.dma_start(lnb[:], lnb_t[:])
            kraw = [
                [p1pool.tile([128, D], dt.bfloat16, tag=f"kr{i}_{m}", name=f"kr{i}_{m}") for m in range(M4)]
                for i in range(2)
            ]

            def pst_tile():
                _n[0] += 1
                return pp_t.tile(
                    [128, 1024], dt.bfloat16, tag="pst", name=f"pst_{_n[0]}"
                )

            def xt8_chunk(m):
                """x^T for token chunk m: 8 transposes + one fp8 cast (ACT)."""
                nc.vector.tensor_copy(xbf[m][:], x32[m][:])
                pst = pst_tile()
                pstv = pst[:].rearrange("p (k t) -> p k t", k=K8)
                for k in range(K8):
                    nc.tensor.transpose(
                        pstv[:, k, :], xbf[m][:, ts(k, 128)], ident_bf[:]
                    )
                nc.scalar.copy(xT8[:, :, ts(m, 128)], pstv[:])

            def extras_chunk(m):
                pse = pp_ex.tile([128, W_EX], dt.float32, tag="pse")
                for g in range(G4):
                    nc.tensor.matmul(
                        pse[:], xT8[:, 2 * g:2 * g + 2, ts(m, 128)], ext[:, g, :, :],
                        start=(g == 0), stop=(g == G4 - 1),
                        perf_mode=PM.DoubleRow,
                    )
                nc.vector.tensor_tensor(exsb[m][:], pse[:], exds[:], ALU.mult)

            def dr_proj(m, augsb, ps):
                """double-row k_proj matmul for token chunk m into psum ps."""
                for g in range(G4):
                    for s0 in (0, 512):
                        nc.tensor.matmul(
                            ps[:, s0:s0 + 512],
                            xT8[:, 2 * g:2 * g + 2, ts(m, 128)],
                            augsb[:, g, :, s0:s0 + 512],
                            start=(g == 0), stop=(g == G4 - 1),
                            perf_mode=PM.DoubleRow,
                        )

            def proj_accums(i, m, augsb, ss, kx, keep):
                """matmul + moment accumulation for one chunk; optionally
                keep k_raw in SBUF (returns the tile or None)."""
                ps = pp_proj.tile([128, D], dt.float32, tag="ps_proj")
                dr_proj(m, augsb, ps)
                scr = scrpool.tile([128, D], dt.bfloat16, tag="scr", name=f"sq_{i}_{m}")
                nc.scalar.activation(scr[:], ps[:], AF.Square, accum_out=ss[:, m:m + 1])
                kr = None
                if keep:
                    kr = kraw[i][m]
                    nc.vector.tensor_copy(kr[:], ps[:])
                scr2 = scrpool.tile([128, D], dt.bfloat16, tag="scr", name=f"kx_{i}_{m}")
                nc.vector.scalar_tensor_tensor(
                    scr2[:], ps[:], 1.0, x32[m][:], ALU.mult, ALU.mult,
                    accum_out=kx[:, m:m + 1],
                )
                return kr

            def o_write(i, m, s, j):
                o = qkpool.tile([128, D], dt.bfloat16, tag="qk", name=f"qk_{i}_{m}")
                nc.vector.scalar_tensor_tensor(
                    o[:], kraw[i][m][:], s[:, j:j + 1], xbf[m][:], ALU.mult, ALU.add
                )
                return o

            def ship_k(m, o):
                """transpose o chunk, cast fp8, DMA into the gather input."""
                pst = pst_tile()
                pstv = pst[:].rearrange("p (k t) -> p k t", k=K8)
                for k in range(K8):
                    nc.tensor.transpose(
                        pstv[:, k, :], o[:, ts(k, 128)], ident_bf[:]
                    )
                strip = kspool.tile([128, K8, 128], SC_DT, tag="kstrip", name=f"kstrip_{m}")
                nc.vector.tensor_copy(strip[:], pstv[:])
                half = agkA_in if m < 2 else agkB_in
                dst = half[0:D, :].rearrange("(k p) t -> p k t", p=128)[:, :, ts(m % 2, 128)]
                nc.sync.dma_start(dst, strip[:])

            # ---------- k path, pipelined per chunk ----------
            ss1, kx1 = sct(scpool, "ss1"), sct(scpool, "kx1")
            for m in range(M4):
                xt8_chunk(m)
                extras_chunk(m)
                proj_accums(1, m, augsbs[1], ss1, kx1, keep=True)
                if m == 0:
                    _, _, s1a = chain(1, ss1, kx1, slice(0, 1), scpool)
                    ship_k(0, o_write(1, 0, s1a, 0))
            _, _, s1b = chain(1, ss1, kx1, slice(1, 4), scpool)
            for m in range(1, M4):
                ship_k(m, o_write(1, m, s1b, m - 1))
                if m == 1:
                    nc.gpsimd.collective_compute(
                        "AllGather", ALU.bypass, ins=[agkA_in[:]],
                        outs=[agkA_out[:]], replica_groups=RG,
                    )

            # ---------- v and q projections interleaved ----------
            ss2, kx2 = sct(scpool, "ss2"), sct(scpool, "kx2")
            ss0, kx0 = sct(scpool, "ss0"), sct(scpool, "kx0")
            for m in range(M4):
                proj_accums(2, m, augsbs[2], ss2, kx2, keep=False)
                proj_accums(0, m, augsbs[0], ss0, kx0, keep=True)
            _, _, s2 = chain(2, ss2, kx2, slice(0, 4), scpool)
            psu = pst_tile()
            for m in range(M4):
                nc.vector.scalar_tensor_tensor(
                    u_bf[m][:], exsb[m][:, EX_B:EX_B + H], s2[:, m:m + 1],
                    exsb[m][:, EX_A:EX_A + H], ALU.mult, ALU.add,
                )
                nc.tensor.transpose(
                    psu[0:H, ts(m, 128)], u_bf[m][:, 0:H], ident_bf[:]
                )
            uTsb = kspool.tile([H, TOK], dt.bfloat16, tag="uT", name="uTsb")
            nc.vector.tensor_copy(uTsb[:], psu[0:H, 0:TOK])
            # uTsb row h = u[t, h] bf16; 128-token quarter jq -> row D+4h+jq
            for jq in range(4):
                nc.sync.dma_start(
                    agkB_in[D + jq:BGD:4, :],
                    uTsb[:, ts(jq, 128)].bitcast(SC_DT),
                )
            nc.gpsimd.collective_compute(
                "AllGather", ALU.bypass, ins=[agkB_in[:]], outs=[agkB_out[:]],
                replica_groups=RG,
            )

            # ---------- q chain + qT (overlaps gather B) ----------
            for m in range(M4):
                nc.vector.tensor_copy(mxs[:, m:m + 1], exsb[m][:, EX_MX:EX_MX + 1])
                nc.vector.tensor_copy(mks3[:, m:m + 1], exsb[m][:, EX_MK3:EX_MK3 + 1])
            _, _, s0 = chain(0, ss0, kx0, slice(0, 4), scpool)
            for m in range(M4):
                o = o_write(0, m, s0, m)
                pst = pst_tile()
                pstv = pst[:].rearrange("p (k t) -> p k t", k=K8)
                for k in range(K8):
                    nc.tensor.transpose(
                        pstv[:, k, :], o[:, ts(k, 128)], ident_bf[:]
                    )
                nc.scalar.copy(qT[:, :, ts(m, 128)], pstv[:])
            for m in range(M4):
                xsq = scrpool.tile([128, D], dt.bfloat16, tag="scr", name=f"xsq_{m}")
                nc.scalar.activation(xsq[:], x32[m][:], AF.Square, accum_out=xxs[:, m:m + 1])

        # =========== phase 2: attention, with delta3 + moments interleaved
        with (
            tc.tile_pool(name="attn_sb", bufs=1) as attn_sb,
            tc.tile_pool(name="epool", bufs=28) as epool,
            tc.tile_pool(name="fin", bufs=2) as fin,
            tc.tile_pool(name="sc2pool", bufs=16) as sc2pool,
            tc.tile_pool(name="scr2", bufs=2) as scr2pool,
            tc.tile_pool(name="pp_sc", bufs=3, space="PSUM") as pp_sc,
            tc.tile_pool(name="pp_nr", bufs=1, space="PSUM") as pp_nr,
            tc.tile_pool(name="pp_aux", bufs=1, space="PSUM") as pp_aux,
        ):
            # kT[k][p, 512c + 128m + t]; halves land independently
            kT = [attn_sb.tile([128, S], SC_DT, tag=f"kT_{k}", name=f"kTsb_{k}") for k in range(K8)]
            srcA = agkA_out[:].rearrange("(c k p) t -> p k c t", c=4, k=K8)
            srcB = agkB_out[:].rearrange("(c r) t -> c r t", c=4)
            srcBk = srcB[:, 0:D, :].rearrange("c (k p) t -> p k c t", k=K8)
            for k in range(K8):
                dst = kT[k][:].rearrange("p (c mt) -> p c mt", c=4)
                nc.sync.dma_start(dst[:, :, 0:2 * 128], srcA[:, k, :, :])
            for k in range(K8):
                dst = kT[k][:].rearrange("p (c mt) -> p c mt", c=4)
                nc.sync.dma_start(dst[:, :, 2 * 128:4 * 128], srcBk[:, k, :, :])

            uext = attn_sb.tile([128, NKC, H, 2], dt.bfloat16, tag="uext")
            nc.vector.memset(uext[:], 1.0)
            # u receive: 64 byte-rows per core (h, jq); jq = chunk m of that
            # core's tokens. Two 128-row tiles, PE-transpose to token-major.
            u_rows = [attn_sb.tile([128, 256], SC_DT, tag=f"u_rows{i}", name=f"u_rows{i}") for i in range(2)]
            for i in range(2):
                for c2 in range(2):
                    nc.sync.dma_start(
                        u_rows[i][c2 * 64:(c2 + 1) * 64, :],
                        srcB[2 * i + c2, D:BGD, :],
                    )

            # half-A key chunks first (kc = 4c + m; m 0,1 from gather A)
            KC_ORDER = [4 * c + m for m in (0, 1) for c in range(4)] + \
                       [4 * c + m for m in (2, 3) for c in range(4)]

            def scores_exp(hp, pos):
                kc = KC_ORDER[pos]
                ps2 = pp_sc.tile([128, 2, TOK], dt.float32, tag="sc2")
                for j in (0, 1):
                    nc.tensor.matmul(
                        ps2[:, j, :],
                        kT[hp][ts(j, 64), ts(kc, 128)],
                        qT[ts(j, 64), hp, :],
                        start=True, stop=True, tile_position=(64 * j, 0),
                    )
                E = epool.tile([128, 2, TOK], dt.bfloat16, tag="E",
                               name=f"E_{hp}_{kc}")
                if pos in DVE_POS:
                    nc.scalar.activation(E[:, 0, :], ps2[:, 0, :], AF.Exp, scale=SCALE)
                    nc.vector.tensor_scalar(
                        E[:, 1, :].bitcast(dt.int16), ps2[:, 1, :],
                        SCHRA_A, SCHRA_B, ALU.mult, ALU.add,
                    )
                else:
                    nc.scalar.activation(E[:], ps2[:], AF.Exp, scale=SCALE)
                return E

            def nr_mm(hp, pos, E, nr_ps):
                kc = KC_ORDER[pos]
                # heads in different PE column halves -> concurrent streams
                for j in (0, 1):
                    nc.tensor.matmul(
                        nr_ps[64 * j:64 * j + 2, :],
                        uext[:, kc, 2 * hp + j, :], E[:, j, :],
                        start=(pos == 0), stop=(pos == NKC - 1),
                        tile_position=(0, 64 * j),
                    )

            def fold(hp, nr_ps):
                """batched n/r fold for this pair: v3acc[:, m] += n/r."""
                nrsb = fin.tile([2, 2, TOK], dt.float32, tag="nrsb", name=f"nrsb_{hp}")
                nc.vector.tensor_copy(nrsb[0:2, 0, :], nr_ps[0:2, :])
                nc.vector.tensor_copy(nrsb[0:2, 1, :], nr_ps[64:66, :])
                psT = pp_aux.tile([128, TOK], dt.float32, tag="aux", name=f"psT_{hp}")
                psTv = psT[:, 0:M4 * 4].rearrange("p (m j) -> p m j", m=M4)
                for m in range(M4):
                    for j in range(2):
                        nc.tensor.transpose(
                            psTv[:, m, 2 * j:2 * j + 2],
                            nrsb[0:2, j, ts(m, 128)], ident_f32[0:2, 0:2],
                        )
                nrT = fin.tile([128, M4, 4], dt.float32, tag="nrTs", name=f"nrT_{hp}")
                nc.vector.tensor_copy(nrT[:], psTv[:])
                rec = fin.tile([128, M4, 2], dt.float32, tag="rec", name=f"rec_{hp}")
                nc.vector.reciprocal(rec[:], nrT[:, :, 1:4:2])
                prod = fin.tile([128, M4, 2], dt.float32, tag="prod", name=f"pr_{hp}")
                nc.vector.tensor_tensor(prod[:], nrT[:, :, 0:4:2], rec[:], ALU.mult)
                pv = fin.tile([128, M4], dt.float32, tag="pv", name=f"pv_{hp}")
                nc.vector.tensor_reduce(pv[:], prod[:], axis=mybir.AxisListType.X, op=ALU.add)
                nc.vector.tensor_tensor(v3acc[:], v3acc[:], pv[:], ALU.add)

            def delta3_half(m, half):
                """delta3 projection half: matmuls + one PSUM->SBUF copy."""
                s0 = 512 * half
                ps = pp_aux.tile([128, 512], dt.float32, tag="aux", name=f"d3_{m}_{s0}")
                for g in range(G4):
                    nc.tensor.matmul(
                        ps[:], xT8[:, 2 * g:2 * g + 2, ts(m, 128)],
                        augsbs[3][:, g, :, s0:s0 + 512],
                        start=(g == 0), stop=(g == G4 - 1),
                        perf_mode=PM.DoubleRow,
                    )
                nc.vector.tensor_copy(k3raw[m][:, s0:s0 + 512], ps[:])

            def delta3_accums(m):
                scr = scr2pool.tile([128, D], dt.bfloat16, tag="scr", name=f"sc3r_{m}")
                nc.vector.scalar_tensor_tensor(
                    scr[:], k3raw[m][:], 1.0, k3raw[m][:], ALU.mult, ALU.mult,
                    accum_out=kks3[:, m:m + 1],
                )
                scr2 = scr2pool.tile([128, D], dt.bfloat16, tag="scr", name=f"sc3r2_{m}")
                nc.vector.scalar_tensor_tensor(
                    scr2[:], k3raw[m][:], 1.0, xbf[m][:], ALU.mult, ALU.mult,
                    accum_out=xks3[:, m:m + 1],
                )
                nc.gpsimd.tensor_tensor(k3g[m][:], k3raw[m][:], lng[:], ALU.mult)

            # ---- hp 0: scores+exps batched first so the late u gather
            # (whose uext copy gates the nr matmuls) can't stall the
            # scores->exp pipeline via queue order.
            E0 = [scores_exp(0, pos) for pos in range(NKC)]
            psu2 = pp_aux.tile([128, 256], dt.bfloat16, tag="aux", name="psu2")
            for i in range(2):
                nc.tensor.transpose(
                    psu2[:, ts(i, 128)], u_rows[i][:].bitcast(dt.bfloat16),
                    ident_bf[:],
                )
            for i in range(2):
                # psu2 col (c2, h, jq) -> uext[p, kc=4(2i+c2)+jq, h]
                dstv = uext[:, 8 * i:8 * (i + 1), :, 0].rearrange(
                    "p (c2 jq) h -> p c2 jq h", c2=2
                )
                srcv = psu2[:, ts(i, 128)].rearrange(
                    "p (c2 h jq) -> p c2 jq h", c2=2, h=H
                )
                nc.vector.tensor_copy(dstv, srcv)
            nr_ps = pp_nr.tile([128, TOK], dt.float32, tag="nr")
            for pos in range(NKC):
                nr_mm(0, pos, E0[pos], nr_ps)
            fold(0, nr_ps)

            # ---- hp 1..7 staggered; delta3/moments in the slack
            for hp in range(1, K8):
                if hp >= 4:
                    nc.gpsimd.tensor_tensor(
                        xg[hp - 4][:], xbf[hp - 4][:], lng[:], ALU.mult
                    )
                if hp == 6:
                    rk3, rr3, _ = chain(3, kks3, xks3, slice(0, 4), sc2pool)
                    nc.vector.tensor_copy(a3s[:], rr3[:, 0:M4])
                    nc.vector.tensor_tensor(b3s[:], rr3[:, 0:M4], rk3[:, 0:M4], ALU.mult)
                nr_ps = pp_nr.tile([128, TOK], dt.float32, tag="nr")
                Es = []
                for pos in range(NKC):
                    Es.append(scores_exp(hp, pos))
                    if pos >= 2:
                        nr_mm(hp, pos - 2, Es[pos - 2], nr_ps)
                # delta3 vector work sits after the exps so it can't delay
                # the DVE exp stream via queue order
                if hp <= 4:
                    delta3_half(hp - 1, 0)
                    delta3_half(hp - 1, 1)
                    delta3_accums(hp - 1)
                nr_mm(hp, NKC - 2, Es[NKC - 2], nr_ps)
                nr_mm(hp, NKC - 1, Es[NKC - 1], nr_ps)
                fold(hp, nr_ps)

            # ---- final delta + layernorm (stats from precomputed moments)
            v3 = fin.tile([128, M4], dt.float32, tag="v3")
            nc.vector.tensor_scalar_add(v3[:], v3acc[:], cvec[:, 7:8])
            s3 = fin.tile([128, M4], dt.float32, tag="s3")
            nc.vector.tensor_tensor(s3[:], v3[:], a3s[:], ALU.mult)
            nc.vector.tensor_tensor(s3[:], s3[:], b3s[:], ALU.subtract)
            # mu = (sum_x + s3*sum_k)/D
            mu = fin.tile([128, M4], dt.float32, tag="mu")
            nc.vector.tensor_tensor(mu[:], s3[:], mks3[:], ALU.mult)
            nc.vector.tensor_tensor(mu[:], mu[:], mxs[:], ALU.add)
            nc.vector.tensor_scalar_mul(mu[:], mu[:], 1.0 / D)
            # E[y^2] = (xx + 2 s3 xk + s3^2 kk)/D ; var = E[y^2] - mu^2
            t1 = fin.tile([128, M4], dt.float32, tag="t1")
            nc.vector.tensor_tensor(t1[:], s3[:], kks3[:], ALU.mult)
            t2 = fin.tile([128, M4], dt.float32, tag="t2")
            nc.vector.tensor_scalar(t2[:], xks3[:], 2.0, None, ALU.mult)
            nc.vector.tensor_tensor(t2[:], t2[:], t1[:], ALU.add)
            nc.vector.tensor_tensor(t2[:], t2[:], s3[:], ALU.mult)
            nc.vector.tensor_tensor(t2[:], t2[:], xxs[:], ALU.add)
            var = fin.tile([128, M4], dt.float32, tag="var")
            nc.vector.tensor_scalar_mul(var[:], t2[:], 1.0 / D)
            mu2 = fin.tile([128, M4], dt.float32, tag="mu2")
            nc.vector.tensor_tensor(mu2[:], mu[:], mu[:], ALU.mult)
            nc.vector.tensor_tensor(var[:], var[:], mu2[:], ALU.subtract)
            nc.vector.tensor_scalar_add(var[:], var[:], LN_EPS)
            lnv2 = fin.tile([128, M4], dt.float32, tag="lnv2")
            nc.scalar.activation(lnv2[:], var[:], AF.Ln)
            rstd = fin.tile([128, M4], dt.float32, tag="rstd")
            nc.scalar.activation(rstd[:], lnv2[:], AF.Exp, scale=-0.5)
            s3r = fin.tile([128, M4], dt.float32, tag="s3r")
            nc.vector.tensor_tensor(s3r[:], s3[:], rstd[:], ALU.mult)
            nmu = fin.tile([128, M4], dt.float32, tag="nmu")
            nc.vector.tensor_scalar_mul(nmu[:], mu[:], -1.0)
            # y = ((xg - mu*lng)*rstd + lnb) + k3g*(s3*rstd)   [g,b folded]
            for m in range(M4):
                w1 = fin.tile([128, D], dt.bfloat16, tag="w1", name=f"w1_{m}")
                nc.vector.scalar_tensor_tensor(
                    w1[:], lng[:], nmu[:, m:m + 1], xg[m][:], ALU.mult, ALU.add
                )
                w2 = fin.tile([128, D], dt.bfloat16, tag="w2", name=f"w2_{m}")
                nc.vector.scalar_tensor_tensor(
                    w2[:], w1[:], rstd[:, m:m + 1], lnb[:], ALU.mult, ALU.add
                )
                yg = fin.tile([128, D], dt.bfloat16, tag="yg", name=f"yg_{m}")
                nc.vector.scalar_tensor_tensor(
                    yg[:], k3g[m][:], s3r[:, m:m + 1], w2[:], ALU.mult, ALU.add
                )
                nc.sync.dma_start(y_t[ts(m, 128), :], yg[:])

    _split_multi_waits(nc)
    nc.finalize()
    return nc


def _fp8_scaled(col):
    """power-of-2 scale putting maxabs near 120; returns (scaled, descale)."""
    m = float(np.max(np.abs(col)))
    if m == 0.0 or not np.isfinite(m):
        return col, 1.0
    sc = 2.0 ** np.floor(np.log2(120.0 / m))
    return col * sc, 1.0 / sc


def _host_prep(inputs):
    """Precompute augmented weights and constants; returns per-core in_maps."""
    f32 = np.float32
    x = np.asarray(inputs["x"], f32)
    Wq, bq = np.asarray(inputs["Wq"], f32), np.asarray(inputs["bq"], f32)
    Wk, bk = np.asarray(inputs["Wk"], f32), np.asarray(inputs["bk"], f32)
    Wv, bv = np.asarray(inputs["Wv"], f32), np.asarray(inputs["bv"], f32)
    Wo, bo = np.asarray(inputs["Wo"], f32), np.asarray(inputs["bo"], f32)
    dWk, dbw = np.asarray(inputs["dWk"], f32), np.asarray(inputs["dbw"], f32)
    dbb, dWv = np.asarray(inputs["dbb"], f32), np.asarray(inputs["dWv"], f32)
    dbv = np.asarray(inputs["dbv"], f32)
    ln_g, ln_b = np.asarray(inputs["ln_g"], f32), np.asarray(inputs["ln_b"], f32)

    w = Wo @ dWv[3]                                   # (D,)
    Wu = np.zeros((D, H), f32)
    for h in range(H):
        Wu[h * HD:(h + 1) * HD, h] = w[h * HD:(h + 1) * HD]
    Bu = (AUG_SCALE * dWk[2]) @ Wu                    # (D, H), pre-scaled

    vw = [Wq @ dWv[0], Wk @ dWv[1], Wv @ dWv[2]]
    vc = [float(bq @ dWv[0] + dbv[0]), float(bk @ dWv[1] + dbv[1]),
          float(bv @ dWv[2] + dbv[2])]
    c3 = float(bo @ dWv[3] + dbv[3])

    fp8 = ml_dtypes.float8_e4m3
    # double-row layout [128, G4, 2, D]: [p, g, j, f] = W[256g+128j+p, f]
    augs = [
        np.ascontiguousarray(
            (AUG_SCALE * dWk[i]).reshape(G4, 2, 128, D).transpose(2, 0, 1, 3)
        ).astype(fp8)
        for i in range(4)
    ]

    ex = np.zeros((D, W_EX), f32)
    for i in range(4):
        ex[:, EX_DBW[i]] = dbw[i]
    for i in range(3):
        ex[:, EX_VW[i]] = vw[i]
    ex[:, EX_A:EX_A + H] = Wu
    ex[:, EX_B:EX_B + H] = Bu
    ex[:, EX_MX] = 1.0
    ex[:, EX_MK3] = (AUG_SCALE * dWk[3]) @ np.ones((D,), f32)
    exds = np.ones((W_EX,), f32)
    for c in range(W_EX):
        ex[:, c], exds[c] = _fp8_scaled(ex[:, c])
    ex8 = np.ascontiguousarray(
        ex.reshape(G4, 2, 128, W_EX).transpose(2, 0, 1, 3)
    ).astype(fp8)
    exds_b = np.broadcast_to(exds[None, :], (128, W_EX)).copy()

    cvec = np.zeros((128, 16), f32)
    for i in range(4):
        cvec[:, i] = -dbb[i]
    for i in range(3):
        cvec[:, 4 + i] = vc[i]
    cvec[:, 7] = c3

    bf = ml_dtypes.bfloat16
    lng = np.broadcast_to(ln_g[None, :], (128, D)).astype(bf).copy()
    lnb = np.broadcast_to(ln_b[None, :], (128, D)).astype(bf).copy()

    xf = x.reshape(B * S, D)
    in_maps = []
    for c in range(N_CORES):
        m = {
            "x": np.ascontiguousarray(xf[c * TOK:(c + 1) * TOK]),
            "ex": ex8, "exds": exds_b, "cvec": cvec, "lng": lng, "lnb": lnb,
        }
        for i in range(4):
            m[f"aug{i}"] = augs[i]
        in_maps.append(m)
    return in_maps


def kernel(**inputs):
    global LAST_RESULTS
    if "nc" not in _CACHE:
        _CACHE["nc"] = _build_program()
    nc = _CACHE["nc"]
    in_maps = _host_prep(inputs)
    res = run_bass_kernel_spmd(nc, in_maps, core_ids=list(range(N_CORES)))
    LAST_RESULTS = res
    out = np.concatenate(
        [np.asarray(res.results[c]["y"]).astype(np.float32) for c in range(N_CORES)],
        axis=0,
    ).reshape(B, S, D)
    return out
